# revision 50
# baseline (speedup 1.0000x reference)
"""Distributed causal multi-head attention for Trainium2 (8 NeuronCores).

Problem (hardcoded): x[2, 2048, 1024], 16 heads, head_dim 64, causal
softmax(QK^T/8)V then out-proj with bias. f32 in/out.

Sharding: data parallel on batch (cores 0-3 -> batch 0, 4-7 -> batch 1),
tensor parallel on heads within each group of 4 (4 heads per core).
Each core:
  - computes Q^T,K^T via fp8(e4m3) DoubleRow matmuls with error-feedback
    weights (wqk_hi + wqk_lo both fp8; x pre-scaled by 8, W by 64), then
    requantizes q,k to fp8 (x1/32 -> 16*q) for the score matmuls
  - scores transposed S^T[k,q] = K Q^T as fp8 DoubleRow matmuls: head_dim
    64 sits in partitions 0-63 with a zero-filled second DR slot, so each
    512-wide score matmul costs half a bf16 one; the softmax denominator
    comes out of the PE via an appended ones-column on V
  - exp without max-subtraction (scores are O(2), safe in fp32/bf16)
  - causal mask applied post-exp as a 0/1 bf16 multiply on the GPSIMD
    (Pool) engine, which is otherwise idle
  - ctx^T accumulated per q-chunk in bf16, normalized with 1/den
    partition-broadcast
  - row-parallel out-proj: each core computes the FULL-width partial
    outT_part[oc, q] = Wo[own 256 rows, oc]^T ctxT_own; per-q-chunk
    batched HWDGE DMA of the staged [128, 8, 512] output block
No collectives: the host sums the 4 partial outputs per batch group
(standard row-parallel TP unshard), adds the bias, and transposes.
"""

import numpy as np
import ml_dtypes

from concourse import bass, bacc, mybir
from concourse import tile
from concourse.bass_utils import run_bass_kernel_spmd

BF16 = mybir.dt.bfloat16
F32 = mybir.dt.float32
FP8 = mybir.dt.float8e4
Act = mybir.ActivationFunctionType
DR = mybir.MatmulPerfMode.DoubleRow

B, S, D = 2, 2048, 1024
H, HD = 16, 64
NCORES = 8
GROUP = 4            # cores per batch group
HPC = H // GROUP     # 4 heads per core
CW = HPC * HD        # 256 ctx columns per core
QC = 512             # q-chunk width
KC = 128             # k-chunk width
NQ = S // QC         # 4
NKC = S // KC        # 16
KPQ = QC // KC       # 4 k-chunks per q-chunk
DCH = D // 128       # 8 contraction chunks of 128
CCH = CW // 128      # 2 own-ctx contraction chunks
OCH = D // 128       # 8 out-column chunks

X_SCALE = 8.0        # host pre-scale of x before fp8 quantization
W_SCALE = 64.0       # host pre-scale of Wq/Wk before fp8 quantization
QK8 = 16.0           # scale of the requantized fp8 q,k
RQ = QK8 / (X_SCALE * W_SCALE)   # PSUM (512q) -> fp8 (16q) multiplier
EXP_SCALE = 0.125 / (QK8 * QK8)  # exp(q.k/8) with (16q).(16k) scores

_CACHE = {}


def _build_bass(reps=1):
    nc = bacc.Bacc(
        "TRN2", target_bir_lowering=False, debug=False, num_devices=NCORES
    )

    # per-core external inputs, pre-staged by the host in SBUF layout
    # [partition, chunk, free] so each is a single large-descriptor DMA
    x8 = nc.declare_dram_parameter("x8", [128, DCH, S], FP8, isOutput=False)
    # fp8 residual of x8 (error feedback for the V projection)
    x8l = nc.declare_dram_parameter("x8l", [128, DCH, S], FP8, isOutput=False)
    # Wq and Wk interleaved in one tensor with their fp8 quantization
    # residuals (error feedback) adjacent, so each qk-half streams as one
    # DMA with 512B-contiguous runs; hi and lo accumulate in one PSUM group
    wqk = nc.declare_dram_parameter("wqk", [128, DCH, 2, 2, CW], FP8, isOutput=False)
    # wv hi/lo fp8 pair (error feedback), same accumulation-group trick
    wv = nc.declare_dram_parameter("wv", [128, DCH, 2, CW], FP8, isOutput=False)
    wo = nc.declare_dram_parameter("wo", [128, CCH, D], BF16, isOutput=False)
    # wo hi/lo fp8 in DoubleRow layout [partition, ctx-chunk slot, hl, D]
    wo8 = nc.declare_dram_parameter("wo8", [128, CCH, 2, D], FP8, isOutput=False)
    msk = nc.declare_dram_parameter("msk", [128, KPQ, QC], BF16, isOutput=False)
    # selector for den broadcast: bc[m,q] = sum_k sel33[k,m]*den_pair[k,q]
    sel33 = nc.declare_dram_parameter("sel33", [33, 128], BF16, isOutput=False)
    # outT[p, o, q] = partial out^T[o*128+p, q] (host restripes)
    outT = nc.declare_dram_parameter("outT", [128, OCH, S], BF16, isOutput=True)

    with tile.TileContext(nc) as tc:
        with tc.tile_pool(name="persist", bufs=1) as pp:
            x8_sb = pp.tile([128, DCH, S], FP8, tag="x8_sb")
            x8l_sb = pp.tile([128, DCH, S], FP8, tag="x8l_sb")
            wqk_sb = pp.tile([128, DCH, 2, 2, CW], FP8, tag="wqk_sb")
            wv_sb = pp.tile([128, DCH, 2, CW], FP8, tag="wv_sb")
            wo_sb = pp.tile([128, CCH, D], BF16, tag="wo_sb")
            wo8_sb = pp.tile([128, CCH, 2, D], FP8, tag="wo8_sb")
            # ctxn hi/lo fp8 in DoubleRow layout: slot = ctx chunk (pair)
            ctxn8h = pp.tile([128, 2, S], FP8, tag="ctxn8h")
            ctxn8l = pp.tile([128, 2, S], FP8, tag="ctxn8l")
            msk_sb = pp.tile([128, KPQ, QC], BF16, tag="msk_sb")
            # fp8 q,k at 16x true scale: [partition(hd), pair, DR slot, q]
            # slot 1 is zero-filled (DMA) so Ki=64 DoubleRow matmuls see a
            # 128-deep contraction with a dead upper half
            qT8 = pp.tile([128, 2, 2, S], FP8, tag="qT8")
            kT8 = pp.tile([128, 2, 2, S], FP8, tag="kT8")
            v_aug = pp.tile([128, NKC, HPC, HD + 1], BF16, tag="v_aug")
            ctxu0 = pp.tile([128, S], F32, tag="ctxu0")
            ctxu1 = pp.tile([128, S], F32, tag="ctxu1")
            ctxn0 = pp.tile([128, S], BF16, tag="ctxn0")
            ctxn1 = pp.tile([128, S], BF16, tag="ctxn1")
            # den per pair: head 2p at partition 0, head 2p+1 at partition
            # 32 (ACT writes must start at multiples of 32); rows 1-31 are
            # zeroed so the K=33 selector matmul can broadcast both heads
            # to output partitions 0-63 / 64-127 in one instruction
            den_pair = [pp.tile([33, S], BF16, tag=f"den{p}", name=f"den{p}")
                        for p in range(2)]
            sel_sb = pp.tile([33, 128], BF16, tag="sel_sb")
            ctxu_pair = [ctxu0, ctxu1]
            ctxn_pair = [ctxn0, ctxn1]
            # Pool engine zeroes the den scratch
            for p in range(2):
                nc.gpsimd.memset(den_pair[p][:], 0.0)

            # DMA order = first-use order. The first x8 window is split
            # per d-chunk so the very first projection chain unblocks
            # after wq + one small chunk; everything else streams behind.
            # All of x8 (fp8, 2MB) lands before xT (bf16, 4MB) so the QK
            # projections and all j=0 scores can run while V streams in.
            def _x8w(w):
                nc.sync.dma_start(
                    x8_sb[:, :, w * QC:(w + 1) * QC],
                    x8[:, :, w * QC:(w + 1) * QC],
                )

            def _x8lw(w):
                nc.sync.dma_start(
                    x8l_sb[:, :, w * QC:(w + 1) * QC],
                    x8l[:, :, w * QC:(w + 1) * QC],
                )

            # ones column of V_aug and the dead DoubleRow slot of qT8/kT8
            # via the idle Pool engine (off the serial DMA stream)
            nc.gpsimd.memset(v_aug[:, :, :, HD:HD + 1], 1.0)
            nc.gpsimd.memset(qT8[:, :, 1, :], 0.0)
            nc.gpsimd.memset(kT8[:, :, 1, :], 0.0)
            # wqk split per qk half (and the Q half per chunk pair) so the
            # first Q chain unblocks as soon as its first operands land
            for c in range(0, DCH, 2):
                nc.sync.dma_start(wqk_sb[:, c:c + 2, 0, :, :],
                                  wqk[:, c:c + 2, 0, :, :])
                nc.sync.dma_start(x8_sb[:, c:c + 2, 0:QC],
                                  x8[:, c:c + 2, 0:QC])
            nc.sync.dma_start(wqk_sb[:, :, 1, :, :], wqk[:, :, 1, :, :])
            for w in range(1, NQ):
                _x8w(w)
            # msk is first consumed by the j=0 mask muls (~15us in): keep it
            # out of the latency-critical x8 stretch
            nc.sync.dma_start(msk_sb[:], msk[:])
            nc.sync.dma_start(wv_sb[:], wv[:])
            nc.sync.dma_start(sel_sb[:], sel33[:])
            for w in range(NQ):
                _x8lw(w)
            nc.sync.dma_start(wo8_sb[:], wo8[:])
            nc.sync.dma_start(wo_sb[:], wo[:])

            def _emit_once():
                with tc.tile_pool(name="proj_ps", bufs=2, space="PSUM") as projp, \
                     tc.tile_pool(name="sc_ps", bufs=2, space="PSUM") as scp, \
                     tc.tile_pool(name="ctbc_ps", bufs=2, space="PSUM") as ctp, \
                     tc.tile_pool(name="es_pool", bufs=26) as esp, \
                     tc.tile_pool(name="out_sb", bufs=2) as outs, \
                     tc.tile_pool(name="norm", bufs=2) as np_pool:

                    def qk_round(j):
                        # Q,K projections for q/k-token window j; fp8
                        # DoubleRow: chunk pairs -> K=256, with the wqk lo
                        # (error-feedback) pass in the same PSUM
                        # accumulation group. The PSUM (512x scale) is then
                        # requantized to fp8 at 16x by the 1/32 DVE mul.
                        for pair in range(2):
                            for qk, dst in ((0, qT8), (1, kT8)):
                                ps = projp.tile([128, QC], F32, tag="proj")
                                for hl in range(2):
                                    for c in range(0, DCH, 2):
                                        nc.tensor.matmul(
                                            ps[:],
                                            wqk_sb[:, c:c + 2, qk, hl,
                                                   pair * 128:(pair + 1) * 128],
                                            x8_sb[:, c:c + 2, j * QC:(j + 1) * QC],
                                            start=(hl == 0 and c == 0),
                                            stop=(hl == 1 and c == DCH - 2),
                                            perf_mode=DR,
                                        )
                                nc.vector.tensor_scalar_mul(
                                    dst[:, pair, 0, j * QC:(j + 1) * QC],
                                    ps[:], RQ,
                                )

                    def qk_round0():
                        # j=0 is on the critical path to the first exp and
                        # is paced by the x8/wqk DMA stream: run all four
                        # (pair, qk) accumulation groups concurrently in
                        # borrowed scores-PSUM slots, interleaved per chunk
                        # pair, so every group finishes with the last DMA
                        # piece instead of serially after it.
                        ts = [scp.tile([128, 2, QC], F32, tag="st",
                                       name=f"qk0_{i}") for i in range(2)]
                        grp = [(p, qk) for p in range(2) for qk in range(2)]
                        ps_of = {g: ts[gi // 2][:, gi % 2, :]
                                 for gi, g in enumerate(grp)}
                        for hl in range(2):
                            for c in range(0, DCH, 2):
                                for p, qk in grp:
                                    nc.tensor.matmul(
                                        ps_of[(p, qk)],
                                        wqk_sb[:, c:c + 2, qk, hl,
                                               p * 128:(p + 1) * 128],
                                        x8_sb[:, c:c + 2, 0:QC],
                                        start=(hl == 0 and c == 0),
                                        stop=(hl == 1 and c == DCH - 2),
                                        perf_mode=DR,
                                    )
                        for p, qk in grp:
                            nc.vector.tensor_scalar_mul(
                                (qT8 if qk == 0 else kT8)[:, p, 0, 0:QC],
                                ps_of[(p, qk)], RQ,
                            )

                    def v_round(w):
                        # V for token chunks 4w..4w+3, all 4 heads at once.
                        # fp8 DoubleRow, 3 error-feedback terms: x8*wv_hi +
                        # x8*wv_lo + x8lo*wv_hi, all in one PSUM group at
                        # (8x)(64Wv) scale; the copy rescales to true V.
                        for t in range(4 * w, 4 * w + 4):
                            ps = projp.tile([128, QC], F32, tag="proj")
                            terms = ((x8_sb, 0), (x8_sb, 1), (x8l_sb, 0))
                            for i, (xs, hl) in enumerate(terms):
                                for c in range(0, DCH, 2):
                                    nc.tensor.matmul(
                                        ps[:, 0:CW],
                                        xs[:, c:c + 2, t * 128:(t + 1) * 128],
                                        wv_sb[:, c:c + 2, hl, :],
                                        start=(i == 0 and c == 0),
                                        stop=(i == 2 and c == DCH - 2),
                                        perf_mode=DR,
                                    )
                            nc.vector.tensor_scalar_mul(
                                v_aug[:, t, :, 0:HD],
                                ps[:, 0:CW].rearrange("p (h w) -> p h w", h=HPC),
                                1.0 / (X_SCALE * W_SCALE),
                            )

                    def scores_exp(h, j):
                        """S^T then exp (+ causal masking) for q-chunk j of
                        head h, via fp8 DoubleRow matmuls (dead upper slot).
                        Off-band k-chunks (fully below the diagonal) get
                        full-width matmuls; the 4-chunk diagonal band uses
                        shrinking q-windows (exact block causality) with a
                        128-wide triangle mask per chunk (on Pool). Returns
                        the es tiles (off-band pairs + band tiles 1, 2)."""
                        pair, hh = h // 2, h % 2
                        row = hh * 64
                        qs = slice(j * QC, (j + 1) * QC)
                        es_tiles = []
                        # off-band: k-chunks 0 .. 4j-1, two per PSUM tile
                        for c0 in range(0, 4 * j, 2):
                            st = scp.tile([128, 2, QC], F32, tag="st")
                            for i in range(2):
                                c = c0 + i
                                nc.tensor.matmul(
                                    st[:, i, :],
                                    kT8[row:row + 64, pair, :, c * KC:(c + 1) * KC],
                                    qT8[row:row + 64, pair, :, qs],
                                    start=True, stop=True,
                                    perf_mode=DR,
                                )
                            es = esp.tile([128, 2, QC], BF16, tag="es")
                            nc.scalar.activation(es[:], st[:], Act.Exp,
                                                 scale=EXP_SCALE)
                            es_tiles.append(es)
                        # diagonal band: k-chunks 4j+r, q-window [128r, 512)
                        # packed as two tiles; sub-window starts snapped so
                        # each exp is one full-AP instruction (the unwritten
                        # PSUM slivers are exp'd but never read)
                        win = [0, KC, 2 * KC, 3 * KC]
                        for ti in range(2):
                            st = scp.tile([128, 2, QC], F32, tag="st")
                            for i in range(2):
                                r = 2 * ti + i
                                w0 = win[r]
                                nc.tensor.matmul(
                                    st[:, i, w0:QC],
                                    kT8[row:row + 64, pair, :,
                                        (4 * j + r) * KC:(4 * j + r + 1) * KC],
                                    qT8[row:row + 64, pair, :,
                                        j * QC + w0:(j + 1) * QC],
                                    start=True, stop=True,
                                    perf_mode=DR,
                                )
                            es = esp.tile([128, 2, QC], BF16, tag="es")
                            lo = win[2 * ti]
                            nc.scalar.activation(es[:, :, lo:QC],
                                                 st[:, :, lo:QC],
                                                 Act.Exp, scale=EXP_SCALE)
                            # triangle mask on the leading 128 q of each
                            # chunk's window (DVE: low latency matters, the
                            # ctx matmuls consume these immediately)
                            for i in range(2):
                                r = 2 * ti + i
                                w0 = win[r]
                                nc.vector.tensor_mul(
                                    es[:, i, w0:w0 + KC], es[:, i, w0:w0 + KC],
                                    msk_sb[:, r, w0:w0 + KC],
                                )
                            es_tiles.append(es)
                        return es_tiles

                    def ctx_acc(h, j, es_tiles):
                        """attn@V accumulation + write-back for (h, j)."""
                        pair, hh = h // 2, h % 2
                        row = hh * 64
                        qs = slice(j * QC, (j + 1) * QC)
                        win = [0, KC, 2 * KC, 3 * KC]
                        ct = ctp.tile([HD + 1, QC], F32, tag="ct")
                        for c in range(4 * j):
                            nc.tensor.matmul(
                                ct[:],
                                v_aug[:, c, h, :],
                                es_tiles[c // 2][:, c % 2, :],
                                start=(c == 0),
                                stop=False,
                            )
                        for r in range(4):
                            w0 = win[r]
                            nc.tensor.matmul(
                                ct[:, w0:QC],
                                v_aug[:, 4 * j + r, h, :],
                                es_tiles[2 * j + r // 2][:, r % 2, w0:QC],
                                start=(j == 0 and r == 0),
                                stop=(r == 3),
                            )
                        if h == 3 and j == NQ - 1:
                            # final head: ACT is idle by now — take both
                            # write-backs off the DVE-serial tail chain,
                            # den first since it gates the bc matmul
                            # (ACT writes must start at multiples of 32)
                            nc.scalar.activation(
                                den_pair[pair][hh * 32:hh * 32 + 1, qs],
                                ct[HD:HD + 1, :], Act.Identity,
                            )
                            nc.scalar.activation(
                                ctxu_pair[pair][row:row + 64, qs],
                                ct[0:HD, :], Act.Identity,
                            )
                        else:
                            nc.vector.tensor_copy(
                                den_pair[pair][hh * 32:hh * 32 + 1, qs],
                                ct[HD:HD + 1, :],
                            )
                            nc.vector.tensor_copy(
                                ctxu_pair[pair][row:row + 64, qs], ct[0:HD, :]
                            )

                    def norm(pair, j):
                        # den[pair][:, qs] complete once both heads of the
                        # pair finished ctx for q-chunk j
                        qs = slice(j * QC, (j + 1) * QC)
                        bc = ctp.tile([128, QC], F32, tag="ct")
                        nc.tensor.matmul(
                            bc[:], sel_sb[:], den_pair[pair][:, qs],
                            start=True, stop=True,
                        )
                        rb = np_pool.tile([128, QC], F32, tag="rb")
                        nc.vector.reciprocal(rb[:], bc[:])
                        if j == NQ - 1:
                            # final window: bf16 out-proj path (short
                            # latency to the kernel tail)
                            nc.vector.tensor_mul(
                                ctxn_pair[pair][:, qs],
                                ctxu_pair[pair][:, qs], rb[:],
                            )
                        else:
                            # fp8 hi/lo path: hi on DVE; the residual on
                            # the idle Pool engine (ctxn8l = ctx - hi),
                            # well off the out_round critical path
                            nc.vector.tensor_mul(
                                ctxn8h[:, pair, qs],
                                ctxu_pair[pair][:, qs], rb[:],
                            )
                            tf = np_pool.tile([128, QC], F32, tag="tf")
                            nc.gpsimd.tensor_mul(
                                tf[:], ctxu_pair[pair][:, qs], rb[:],
                            )
                            nc.gpsimd.tensor_sub(
                                ctxn8l[:, pair, qs], tf[:],
                                ctxn8h[:, pair, qs],
                            )

                    def out_round(j, last=False):
                        # row-parallel out-proj for q-chunk j:
                        # outT_part[oc, q] = Wo[own, oc]^T ctxn_own (bias on
                        # host). All 8 o-chunks stage into one SBUF block,
                        # then a single batched HWDGE DMA ships the block.
                        # While interleaved into late attention ACT is
                        # exp-critical, so copies go to DVE; the final round
                        # alternates ACT/DVE.
                        qs = slice(j * QC, (j + 1) * QC)
                        ob = outs.tile([128, OCH, QC], BF16, tag="ot")
                        ps2 = None
                        for o in range(OCH):
                            if last and o % 4 < 2:
                                # final round: the scores PSUM is free, so
                                # widen the accumulator set (6 in flight)
                                # to keep the matmuls ahead of the copies
                                if o % 4 == 0:
                                    ps2 = scp.tile([128, 2, QC], F32,
                                                   tag="st", name=f"eps{o}")
                                ps = ps2[:, o % 4, :]
                            else:
                                ps = projp.tile([128, QC], F32, tag="proj")
                            oc = slice(o * 128, (o + 1) * 128)
                            if last:
                                for c in range(CCH):
                                    nc.tensor.matmul(
                                        ps[:],
                                        wo_sb[:, c, oc],
                                        ctxn_pair[c][:, qs],
                                        start=(c == 0),
                                        stop=(c == CCH - 1),
                                    )
                            else:
                                # fp8 DoubleRow with 3 error-feedback
                                # terms: Whi*Chi + Whi*Clo + Wlo*Chi
                                for i, (hl, cx) in enumerate(
                                        ((0, ctxn8h), (0, ctxn8l),
                                         (1, ctxn8h))):
                                    nc.tensor.matmul(
                                        ps[:],
                                        wo8_sb[:, :, hl, oc],
                                        cx[:, :, qs],
                                        start=(i == 0),
                                        stop=(i == 2),
                                        perf_mode=DR,
                                    )
                            if last and o % 2 == 0:
                                # bf16 path accumulated Wo*(8*ctx)
                                nc.scalar.activation(
                                    ob[:, o, :], ps[:], Act.Identity,
                                    scale=0.125,
                                )
                            elif last:
                                nc.vector.tensor_scalar_mul(
                                    ob[:, o, :], ps[:], 0.125,
                                )
                            else:
                                # fp8 path accumulated (16*Wo)*(8*ctx)
                                nc.vector.tensor_scalar_mul(
                                    ob[:, o, :], ps[:], 1.0 / 128.0,
                                )
                            if last and o % 2 == 1:
                                # final round: ship o-pairs as they finish
                                # to keep the post-compute DMA tail short
                                nc.sync.dma_start(
                                    outT[:, o - 1:o + 1, qs],
                                    ob[:, o - 1:o + 1, :],
                                )
                        if not last:
                            nc.sync.dma_start(outT[:, :, qs], ob[:])

                    # Interleaved emission. QK projections and all j=0
                    # scores depend only on x8 (fp8, lands first), so they
                    # run while the larger bf16 x for V is still streaming;
                    # exp starts ~15us earlier than a phase-ordered kernel.
                    # The attention pipeline emits scores of the next chunk
                    # before ctx of the previous one so the PE has queued
                    # work while ACT runs exp; each (pair, j) normalizes as
                    # soon as its den is complete.
                    qk_round(0)
                    es00 = scores_exp(0, 0)
                    es10 = scores_exp(1, 0)
                    qk_round(1)
                    es20 = scores_exp(2, 0)
                    es30 = scores_exp(3, 0)
                    qk_round(2)
                    # j=1 scores pulled ahead of the V projections so ACT
                    # has exp queued through the projection-heavy stretch
                    es01 = scores_exp(0, 1)
                    es11 = scores_exp(1, 1)
                    qk_round(3)
                    es21 = scores_exp(2, 1)
                    v_round(0)
                    ctx_acc(0, 0, es00)
                    ctx_acc(1, 0, es10)
                    norm(0, 0)
                    es31 = scores_exp(3, 1)
                    v_round(1)
                    ctx_acc(2, 0, es20)
                    ctx_acc(3, 0, es30)
                    norm(1, 0)
                    es02 = scores_exp(0, 2)
                    ctx_acc(0, 1, es01)
                    es12 = scores_exp(1, 2)
                    v_round(2)
                    ctx_acc(1, 1, es11)
                    norm(0, 1)
                    es22 = scores_exp(2, 2)
                    v_round(3)
                    ctx_acc(2, 1, es21)
                    ctx_acc(3, 1, es31)
                    norm(1, 1)
                    es32 = scores_exp(3, 2)
                    ctx_acc(0, 2, es02)
                    es03 = scores_exp(0, 3)
                    ctx_acc(1, 2, es12)
                    norm(0, 2)
                    es13 = scores_exp(1, 3)
                    ctx_acc(2, 2, es22)
                    out_round(0)
                    es23 = scores_exp(2, 3)
                    ctx_acc(3, 2, es32)
                    norm(1, 2)
                    es33 = scores_exp(3, 3)
                    ctx_acc(0, 3, es03)
                    out_round(1)
                    ctx_acc(1, 3, es13)
                    norm(0, 3)
                    ctx_acc(2, 3, es23)
                    # tail-critical chain outranks out_round(2) in the
                    # scheduler's priority heap; out_round(2) fills PE gaps
                    ctx_acc(3, 3, es33)
                    norm(1, 3)
                    out_round(2)
                    out_round(3, last=True)

            for _rep in range(reps):
                _emit_once()
    nc.compile()
    return nc


def _causal_mask():
    # msk[kp, r, qf] = 1 where (r*128 + kp) <= qf else 0  (keep k <= q)
    kp = np.arange(128)[:, None, None]
    r = np.arange(KPQ)[None, :, None]
    qf = np.arange(QC)[None, None, :]
    return (r * 128 + kp <= qf).astype(ml_dtypes.bfloat16)


def _stage(a, dtype, pdim=128):
    """[pdim*n, free...] -> contiguous [pdim, n, free...]"""
    n = a.shape[0] // pdim
    out = a.reshape((n, pdim) + a.shape[1:]).transpose(
        (1, 0) + tuple(range(2, a.ndim + 1))
    )
    return np.ascontiguousarray(out.astype(dtype))


def _in_maps(x, Wq, Wk, Wv, Wo, bo):
    bf = ml_dtypes.bfloat16
    f8 = ml_dtypes.float8_e4m3
    msk = _causal_mask()
    # 1/8 folded in: rb = 8/den, so ctxn = 8*ctx sits in fp8's sweet spot
    sel33 = np.zeros((33, 128), dtype=bf)
    sel33[0, 0:64] = 0.125
    sel33[32, 64:128] = 0.125
    xTs = [np.ascontiguousarray(x[b].T) for b in range(B)]
    x8s, x8ls = [], []
    for xb in xTs:
        hi = (xb * X_SCALE).astype(f8)
        lo = xb * X_SCALE - hi.astype(np.float32)
        x8s.append(_stage(hi.astype(np.float32), f8))
        x8ls.append(_stage(lo, f8))

    def _hilo(w):
        hi = (w * W_SCALE).astype(f8).astype(np.float32)
        lo = w * W_SCALE - hi
        return hi, lo

    maps = []
    for c in range(NCORES):
        b, g = c // GROUP, c % GROUP
        cs = slice(g * CW, (g + 1) * CW)
        wq_hi, wq_lo = _hilo(Wq[:, cs])
        wk_hi, wk_lo = _hilo(Wk[:, cs])
        wv_hi, wv_lo = _hilo(Wv[:, cs])
        wocs = np.ascontiguousarray(Wo[cs, :])
        wo8_hi = (wocs * 16.0).astype(f8).astype(np.float32)
        wo8_lo = wocs * 16.0 - wo8_hi
        maps.append({
            "x8": x8s[b],
            "x8l": x8ls[b],
            # [128, DCH, qk, hl, CW]
            "wqk": np.ascontiguousarray(np.stack([
                np.stack([_stage(wq_hi, f8), _stage(wq_lo, f8)], axis=2),
                np.stack([_stage(wk_hi, f8), _stage(wk_lo, f8)], axis=2),
            ], axis=2)),
            "wv": np.ascontiguousarray(np.stack(
                [_stage(wv_hi, f8), _stage(wv_lo, f8)], axis=2)),
            "wo": _stage(wocs, bf),
            "wo8": np.ascontiguousarray(np.stack(
                [_stage(wo8_hi, f8), _stage(wo8_lo, f8)], axis=2)),
            "msk": msk,
            "sel33": sel33,
        })
    return maps


def kernel(x, Wq, Wk, Wv, Wo, bo, _trace=False):
    x = np.asarray(x, dtype=np.float32)
    Wq, Wk, Wv, Wo, bo = (np.asarray(a, dtype=np.float32) for a in (Wq, Wk, Wv, Wo, bo))
    if "nc" not in _CACHE:
        _CACHE["nc"] = _build_bass()
    nc = _CACHE["nc"]
    res = run_bass_kernel_spmd(
        nc, _in_maps(x, Wq, Wk, Wv, Wo, bo), list(range(NCORES)), trace=_trace
    )
    out = np.zeros((B, S, D), dtype=np.float32)
    for b in range(B):
        acc = np.zeros((D, S), dtype=np.float32)
        for g in range(GROUP):
            # outT[p, o, q] -> [o*128+p, q]
            part = res.results[GROUP * b + g]["outT"].astype(np.float32)
            acc += part.transpose(1, 0, 2).reshape(D, S)
        out[b] = acc.T + bo[None, :]
    if _trace:
        return out, res
    return out
